# revision 36
# baseline (speedup 1.0000x reference)
"""Trainium2 Bass kernel for CachedMultiHeadAttention.

Problem: B=16, Q=32, KV=4096, D=1024, H=16 (DH=64), fp32 in/out.

Sharding: tensor-parallel over heads — each of the 8 cores owns 2 heads
(a contiguous 128-wide slice of D) for ALL 16 batches:
  - Wq/Wk/Wv column-sliced [1024, 128], Wo row-sliced [128, 1024],
    KV cache head-sliced [16, 4096, 128]; x replicated.
  - Each core computes a full-size partial y = wv_c @ Wo_c (+ bo/8);
    a ReduceScatter over the 8 cores sums the partials and hands core c
    tokens 64c..64c+64 (= batches 2c, 2c+1), which is exactly its output
    slice. Host-side work is pure slicing + concatenation.

Per-core dataflow (DMA-bound: ~202 µs of modeled 360 GB/s bandwidth):
  - KV cache streams from t=0 as fp16 via casting gpsimd (SWDGE) DMAs in
    4-stripe units (one 3-dim DMA per batch per tensor, 512 B src lines).
    5-unit tile rings give ~50 µs of prefetch slack.
  - Attention is fully 16-bit with scores built DIRECTLY in transposed
    orientation: stationary K^T chunk [dims, s] x moving block-diagonal
    q [dims, (2h x 32q)] -> scores^T [s, q-blocks]; exp straight off
    PSUM emits w^T; W@V runs with stationary NATURAL V [s, vd] x moving
    w^T -> o^T [vd, q-blocks], which is already the layout the output
    projection wants. No separate w transpose, no V shuffling.
  - Softmax denominators accumulate into 2 spare columns of the o^T
    accumulator via ones-moving matmuls (per-head row sums of w^T).
  - The stripe loop is software-pipelined: K^T transposes run one stripe
    ahead of QK, two ahead of W@V, so PE never waits on the Act/DVE
    copies between stages and the single DMA resource never idles.
  - Per batch-pair extraction: o^T -> (transpose) -> per-(head,q)
    normalize by reciprocal denominators -> (transpose back) -> wv^T.
  - Output projection per 128-token chunk: stationary wv^T x moving Wo
    row-slice (f32r N=512), bias via K=1 ones matmul; partial y goes to
    Internal DRAM scratch in bf16 so the ReduceScatter moves half the
    bytes; the rs output bounces through SBUF to the f32 output.
"""

import numpy as np

import concourse.bass as bass
import concourse.bacc as bacc
import concourse.mybir as mybir
import concourse.tile as tile
from concourse.bass_utils import run_bass_kernel_spmd
from concourse.masks import make_identity

F32 = mybir.dt.float32
F32R = mybir.dt.float32r
BF16 = mybir.dt.bfloat16
FP16 = mybir.dt.float16

B, Q, KV, D, H = 16, 32, 4096, 1024, 16
DH = D // H                     # 64
NCORES = 8
DSL = D // NCORES               # 128: per-core head-slice width (2 heads)
TOK = B * Q                     # 512 tokens, all batches
NBP = B // 2                    # 8 batch pairs
NSTRIPE = 8                     # stripes of 512 cached positions
STRIPE = 512
SCALE = float(DH) ** -0.5       # folded q*k scale (DH**-0.25 applied twice)
PREF = 5                        # KV prefetch ring depth in 4-stripe units
NO = 132                        # o^T accumulator: 128 o cols + 2 den + 2 pad


def _build_kernel():
    nc = bacc.Bacc(
        "TRN2",
        target_bir_lowering=False,
        debug=False,
        enable_asserts=False,
        num_devices=NCORES,
        dynamic_dma_scratch_size=32768,
    )

    x_d = nc.dram_tensor("x", [TOK, D], F32, kind="ExternalInput").ap()
    # KV shards arrive in [b, p, j, d] block layout (pure permutation done
    # while slicing on the host): DMA source runs are 16*128*4 B, so each
    # 4-stripe load is 128 descriptors instead of 1024 and the SWDGE ring
    # never throttles descriptor generation.
    ck_d = nc.dram_tensor("cache_k", [B, 128, KV // 128, DSL], F32,
                          kind="ExternalInput").ap()
    cv_d = nc.dram_tensor("cache_v", [B, 128, KV // 128, DSL], F32,
                          kind="ExternalInput").ap()
    wq_d = nc.dram_tensor("Wq", [D, DSL], F32, kind="ExternalInput").ap()
    wk_d = nc.dram_tensor("Wk", [D, DSL], F32, kind="ExternalInput").ap()
    wv_d = nc.dram_tensor("Wv", [D, DSL], F32, kind="ExternalInput").ap()
    wo_d = nc.dram_tensor("Wo", [DSL, D], F32R, kind="ExternalInput").ap()
    bq_d = nc.dram_tensor("bq", [DSL], F32R, kind="ExternalInput").ap()
    bv_d = nc.dram_tensor("bv", [DSL], F32R, kind="ExternalInput").ap()
    bo_d = nc.dram_tensor("bo", [D], F32, kind="ExternalInput").ap()
    scr_d = nc.dram_tensor("scratch", [TOK, D], BF16, kind="Internal").ap()
    rsb_d = nc.dram_tensor("rs_b", [TOK // NCORES, D], BF16, kind="Internal").ap()
    y_d = nc.dram_tensor("y", [TOK // NCORES, D], F32, kind="ExternalOutput").ap()

    with tile.TileContext(nc) as tc:
        _body(tc, x_d, ck_d, cv_d, wq_d, wk_d, wv_d, wo_d, bq_d, bv_d, bo_d,
              scr_d, rsb_d, y_d)
    nc.compile()
    return nc


def _body(tc, x_d, ck_d, cv_d, wq_d, wk_d, wv_d, wo_d, bq_d, bv_d, bo_d,
          scr_d, rsb_d, y_d):
    nc = tc.nc
    Exp = mybir.ActivationFunctionType.Exp

    ck_r = ck_d.rearrange("b p j d -> p b j d")  # [128, 16, 32, 128]
    cv_r = cv_d.rearrange("b p j d -> p b j d")

    with (
        tc.tile_pool(name="consts", bufs=1) as consts,
        tc.tile_pool(name="knat", bufs=PREF) as knat_p,
        tc.tile_pool(name="vnat", bufs=PREF) as vnat_p,
        tc.tile_pool(name="ktp", bufs=8) as kt_p,
        tc.tile_pool(name="wtp", bufs=8) as wt_p,
        tc.tile_pool(name="work", bufs=3) as work,
        tc.tile_pool(name="ysb", bufs=2) as ysb_p,
        tc.tile_pool(name="fin", bufs=1) as fin,
    ):
        ident_h = consts.tile([128, 128], FP16)
        make_identity(nc, ident_h)
        ident_f = consts.tile([128, 128], F32)
        make_identity(nc, ident_f)
        ident_r = consts.tile([128, 128], F32R)
        nc.vector.tensor_copy(ident_r, ident_f)
        ones2_h = consts.tile([128, 2], FP16)
        nc.vector.memset(ones2_h, 1.0)
        ones_tok_r = consts.tile([1, TOK], F32R)
        nc.vector.memset(ones_tok_r.bitcast(F32), 1.0)
        ones128_r = consts.tile([1, 128], BF16)
        nc.vector.memset(ones128_r, 1.0)

        bq_sb = consts.tile([1, DSL], F32R)
        bv_sb = consts.tile([1, DSL], F32R)
        bo_sb = consts.tile([1, D], F32)
        bo8_sb = consts.tile([1, D], F32R)          # bo / NCORES
        wo_sb = consts.tile([128, D], BF16)

        # stage-A persistent outputs
        xT = consts.tile([128, 8, TOK], FP16)       # [d-part, k-chunk, tok]
        qbd = consts.tile([128, B, DH], FP16)       # block-diag q per batch
        kTc = consts.tile([128, TOK], FP16)         # current-token K^T
        v_cur = consts.tile([Q, B, DSL], FP16)      # current-token V, natural
        nc.vector.memset(qbd, 0.0)                  # off-diag blocks stay 0
        wvT = consts.tile([128, TOK], BF16)         # normalized attn out^T

        # stage-A inputs; casting fp16 loads via gpsimd. The first KV unit
        # is issued ahead of x/W (see main loop) so the DMA engines start on
        # the big stream immediately; wo (f32r, non-cast) rides the scalar
        # queue.
        x_sb = consts.tile([128, 4, D], FP16)
        nc.scalar.dma_start(out=bq_sb, in_=bq_d.rearrange("(a d) -> a d", a=1))
        nc.scalar.dma_start(out=bv_sb, in_=bv_d.rearrange("(a d) -> a d", a=1))
        nc.scalar.dma_start(out=bo_sb, in_=bo_d.rearrange("(a d) -> a d", a=1))
        nc.vector.tensor_scalar_mul(bo8_sb, bo_sb, 1.0 / NCORES)
        bo8_h = consts.tile([1, D], BF16)
        nc.vector.tensor_copy(bo8_h, bo8_sb.bitcast(F32))
        nc.gpsimd.dma_start(out=wo_sb, in_=wo_d)
        wq_h = consts.tile([128, 8, DSL], FP16)
        wk_h = consts.tile([128, 8, DSL], FP16)
        wv_h = consts.tile([128, 8, DSL], FP16)
        nc.gpsimd.dma_start(out=wq_h, in_=wq_d.rearrange("(c p) d -> p c d", p=128))
        nc.gpsimd.dma_start(out=wk_h, in_=wk_d.rearrange("(c p) d -> p c d", p=128))
        nc.gpsimd.dma_start(out=wv_h, in_=wv_d.rearrange("(c p) d -> p c d", p=128))

        # first KV unit, issued ahead of x/W so the stream starts at once
        k_nat0 = knat_p.tile([128, 2, 16, DSL], FP16, tag="knat", name="kn_0_0")
        v_nat0 = vnat_p.tile([128, 2, 16, DSL], FP16, tag="vnat", name="vn_0_0")
        for lo in range(0, 16, 8):
            for half, b in enumerate((0, 1)):
                nc.gpsimd.dma_start(out=k_nat0[:, half, lo : lo + 8, :],
                                    in_=ck_r[:, b, lo : lo + 8, :])
                nc.gpsimd.dma_start(out=v_nat0[:, half, lo : lo + 8, :],
                                    in_=cv_r[:, b, lo : lo + 8, :])
        nc.gpsimd.dma_start(out=x_sb, in_=x_d.rearrange("(c p) d -> p c d", p=128))

        # ---------------- stage A ----------------
        with tc.tile_pool(name="apsum", bufs=2, space="PSUM") as apsum:
            # x^T: per k-chunk, 4 fp16 transposes into one PSUM tile, 1 copy.
            for k in range(8):
                xt_ps = apsum.tile([128, 4, 128], FP16, tag="xt")
                for tch in range(4):
                    nc.tensor.matmul(
                        xt_ps[:, tch, :], x_sb[:, tch, 128 * k : 128 * k + 128],
                        ident_h, start=True, stop=True, is_transpose=True,
                    )
                if k % 2 == 0:
                    nc.scalar.copy(out=xT[:, k, :],
                                   in_=xt_ps.rearrange("p a b -> p (a b)"))
                else:
                    nc.vector.tensor_copy(xT[:, k, :],
                                          xt_ps.rearrange("p a b -> p (a b)"))

            # q^T [128 dims, 512 tok] (+bq), scattered to block-diag qbd
            qt_ps = apsum.tile([128, TOK], F32, tag="proj")
            for k in range(8):
                nc.tensor.matmul(qt_ps, wq_h[:, k, :], xT[:, k, :],
                                 start=(k == 0), stop=False)
            nc.tensor.matmul(qt_ps, bq_sb, ones_tok_r, start=False, stop=True,
                             skip_group_check=True)
            for b in range(B):
                if b % 2 == 0:
                    nc.scalar.copy(out=qbd[0:64, b, 0:Q],
                                   in_=qt_ps[0:64, Q * b : Q * b + Q])
                    nc.scalar.copy(out=qbd[64:128, b, Q : 2 * Q],
                                   in_=qt_ps[64:128, Q * b : Q * b + Q])
                else:
                    nc.vector.tensor_copy(qbd[0:64, b, 0:Q],
                                          qt_ps[0:64, Q * b : Q * b + Q])
                    nc.vector.tensor_copy(qbd[64:128, b, Q : 2 * Q],
                                          qt_ps[64:128, Q * b : Q * b + Q])

            # k_cur^T [128 dims, 512 tok]
            kt_ps = apsum.tile([128, TOK], F32, tag="proj")
            for k in range(8):
                nc.tensor.matmul(kt_ps, wk_h[:, k, :], xT[:, k, :],
                                 start=(k == 0), stop=(k == 7))
            nc.scalar.copy(out=kTc, in_=kt_ps)

            # v_cur per batch, natural [32 tok, 128 dims] on partitions 0:32
            # (stationary for the current-token W@V), bp-ascending order.
            for bb in range(B // 4):
                vn_ps = apsum.tile([Q, 4, DSL], F32, tag="vn")
                for i in range(4):
                    b = 4 * bb + i
                    for k in range(8):
                        nc.tensor.matmul(
                            vn_ps[:, i, :], xT[:, k, Q * b : Q * b + Q],
                            wv_h[:, k, :], start=(k == 0), stop=False)
                    nc.tensor.matmul(
                        vn_ps[:, i, :], ones_tok_r[:, 0:Q], bv_sb,
                        start=False, stop=True, skip_group_check=True)
                    dst = v_cur[:, b, :]
                    if i % 2 == 0:
                        nc.scalar.copy(out=dst, in_=vn_ps[:, i, :])
                    else:
                        nc.vector.tensor_copy(dst, vn_ps[:, i, :])

        # ---------------- main attention loop ----------------
        with (
            tc.tile_pool(name="trpsum", bufs=3, space="PSUM") as trpsum,
            tc.tile_pool(name="scpsum", bufs=2, space="PSUM") as scpsum,
            tc.tile_pool(name="opsum", bufs=2, space="PSUM") as opsum,
            tc.tile_pool(name="mpsum", bufs=1, space="PSUM") as mpsum,
        ):
            o_tiles = {}
            kt_tiles = {}
            wt_tiles = {}
            knat_tiles = {(0, 0): k_nat0}
            vnat_tiles = {(0, 0): v_nat0}

            def load_unit(bp, ss):
                """Casting DMAs for 4 stripes x both batches of a pair. The
                globally-last unit loads per-stripe so the pipeline drain
                after the final bytes is one stripe deep, not four."""
                k_nat = knat_p.tile([128, 2, 16, DSL], FP16, tag="knat",
                                    name=f"kn_{bp}_{ss}")
                v_nat = vnat_p.tile([128, 2, 16, DSL], FP16, tag="vnat",
                                    name=f"vn_{bp}_{ss}")
                if not (bp == NBP - 1 and ss == 1):
                    for half, b in enumerate((2 * bp, 2 * bp + 1)):
                        nc.gpsimd.dma_start(
                            out=k_nat[:, half, :, :],
                            in_=ck_r[:, b, 16 * ss : 16 * ss + 16, :])
                        nc.gpsimd.dma_start(
                            out=v_nat[:, half, :, :],
                            in_=cv_r[:, b, 16 * ss : 16 * ss + 16, :])
                else:
                    # last unit: all K first (scores+exp finish before the
                    # final V bytes), then V per-stripe for a short drain.
                    for half, b in enumerate((2 * bp, 2 * bp + 1)):
                        nc.gpsimd.dma_start(
                            out=k_nat[:, half, :, :],
                            in_=ck_r[:, b, 16 * ss : 16 * ss + 16, :])
                    for lo in range(0, 16, 4):
                        for half, b in enumerate((2 * bp, 2 * bp + 1)):
                            nc.gpsimd.dma_start(
                                out=v_nat[:, half, lo : lo + 4, :],
                                in_=cv_r[:, b, 16 * ss + lo : 16 * ss + lo + 4, :])
                knat_tiles[(bp, ss)] = k_nat
                vnat_tiles[(bp, ss)] = v_nat

            def alloc_o(bp):
                o_ps = opsum.tile([128, NO], F32, tag="o", name=f"o_bp{bp}")
                o_tiles[bp] = o_ps

            def current_token(bp, opener=False):
                """Accumulation into o_ps[bp]: normally last (stop=True);
                for the final pair it runs first and opens the bank."""
                o_ps = o_tiles[bp]
                scc = mpsum.tile([Q, 128], F32, tag="m", name=f"scc{bp}")
                for half, b in enumerate((2 * bp, 2 * bp + 1)):
                    nc.tensor.matmul(
                        scc[:, 64 * half : 64 * half + 64],
                        kTc[:, Q * b : Q * b + Q], qbd[:, b, :],
                        start=True, stop=True,
                    )
                wtc = work.tile([Q, 128], FP16, tag="wtc")
                nc.scalar.activation(wtc, scc, Exp, scale=SCALE)
                for half, b in enumerate((2 * bp, 2 * bp + 1)):
                    nc.tensor.matmul(
                        o_ps[:, 64 * half : 64 * half + 64],
                        v_cur[:, b, :], wtc[:, 64 * half : 64 * half + 64],
                        start=(opener and half == 0), stop=not opener,
                        skip_group_check=True,
                    )
                nc.tensor.matmul(
                    o_ps[:, 128:130], wtc, ones2_h[0:Q, :],
                    start=False, stop=not opener, skip_group_check=True,
                )

            def ktr(bp, S):
                """PE-transpose one stripe of K (both batches) into kt via a
                single fp16 PSUM bank and one copy."""
                k_nat = knat_tiles[(bp, S // 4)]
                s4 = S % 4
                kt = kt_p.tile([128, 2, STRIPE], FP16, tag="kt",
                               name=f"kt_{bp}_{S}")
                tr_ps = trpsum.tile([128, 2, 4, 128], FP16, tag="tr")
                for half in range(2):
                    for jj in range(4):
                        nc.tensor.matmul(
                            tr_ps[:, half, jj, :], k_nat[:, half, 4 * s4 + jj, :],
                            ident_h, start=True, stop=True, is_transpose=True,
                        )
                nc.vector.tensor_copy(
                    kt.rearrange("p a b -> p (a b)"),
                    tr_ps.rearrange("p a b c -> p (a b c)"))
                kt_tiles[(bp, S)] = kt
                return kt

            def qk(bp, S):
                """scores^T (+ denominator columns) for a stripe."""
                kt = kt_tiles.pop((bp, S))
                sc = scpsum.tile([128, 4, 128], F32, tag="sc")
                for jj in range(4):
                    for half, b in enumerate((2 * bp, 2 * bp + 1)):
                        nc.tensor.matmul(
                            sc[:, jj, 64 * half : 64 * half + 64],
                            kt[:, half, 128 * jj : 128 * jj + 128],
                            qbd[:, b, :],
                            start=True, stop=True,
                        )
                wt = wt_p.tile([128, 4, 128], FP16, tag="wt",
                               name=f"wt_{bp}_{S}")
                nc.scalar.activation(
                    wt.rearrange("p a b -> p (a b)"),
                    sc.rearrange("p a b -> p (a b)"),
                    Exp, scale=SCALE)
                # denominator accumulation depends only on K: issue it here
                # so the post-stream drain does not serialize behind it.
                o_ps = o_tiles[bp]
                for jj in range(4):
                    nc.tensor.matmul(
                        o_ps[:, 128:130], wt[:, jj, :], ones2_h,
                        start=(bp != NBP - 1 and S == 0 and jj == 0),
                        stop=(bp == NBP - 1 and S == NSTRIPE - 1 and jj == 3),
                        skip_group_check=True,
                    )
                wt_tiles[(bp, S)] = wt
                return wt

            def wv(bp, S):
                """o^T += V_nat-stationary stripe."""
                o_ps = o_tiles[bp]
                v_nat = vnat_tiles[(bp, S // 4)]
                s4 = S % 4
                wt = wt_tiles.pop((bp, S))
                jjs = range(4)
                # exactly ONE start per o_ps bank (start marks the whole 2KB
                # zero-region pending; other regions' first writes land fresh
                # via the pending flag and must NOT re-assert start). For the
                # final pair the opener is the early current-token matmul.
                closer_here = bp == NBP - 1 and S == NSTRIPE - 1
                for jj in jjs:
                    for half in range(2):
                        nc.tensor.matmul(
                            o_ps[:, 64 * half : 64 * half + 64],
                            v_nat[:, half, 4 * s4 + jj, :],
                            wt[:, jj, 64 * half : 64 * half + 64],
                            start=False,
                            stop=(closer_here and jj == 3),
                            skip_group_check=True,
                        )

            def extract(bp):
                """normalize o^T by per-(head,q) denominators -> wvT."""
                o_ps = o_tiles.pop(bp)
                o_sb = work.tile([128, 128], F32R, tag="osb")
                nc.scalar.copy(out=o_sb, in_=o_ps[:, 0:128])
                rec = work.tile([128, 1], F32, tag="rec")
                nc.vector.reciprocal(rec, o_ps[:, 128:129])
                t2 = mpsum.tile([128, 128], F32R, tag="m", name=f"t2_{bp}")
                nc.tensor.matmul(t2, o_sb, ident_r,
                                 start=True, stop=True, is_transpose=True)
                onrm = work.tile([128, 128], F32R, tag="onrm")
                nc.vector.tensor_scalar_mul(onrm, t2, rec)
                t3 = mpsum.tile([128, 128], F32R, tag="m", name=f"t3_{bp}")
                nc.tensor.matmul(t3, onrm, ident_r,
                                 start=True, stop=True, is_transpose=True)
                for half, b in enumerate((2 * bp, 2 * bp + 1)):
                    if half == 0:
                        nc.scalar.copy(
                            out=wvT[0:64, Q * b : Q * b + Q],
                            in_=t3[0:64, 64 * half : 64 * half + Q])
                        nc.scalar.copy(
                            out=wvT[64:128, Q * b : Q * b + Q],
                            in_=t3[64:128, 64 * half + Q : 64 * half + 2 * Q])
                    else:
                        nc.vector.tensor_copy(
                            wvT[0:64, Q * b : Q * b + Q],
                            t3[0:64, 64 * half : 64 * half + Q])
                        nc.vector.tensor_copy(
                            wvT[64:128, Q * b : Q * b + Q],
                            t3[64:128, 64 * half + Q : 64 * half + 2 * Q])

            def oproj(tch):
                """output projection + bias for one 128-token chunk."""
                y_sb = ysb_p.tile([128, D], BF16, tag="ysb")
                for half in range(2):
                    yp = mpsum.tile([128, 512], F32, tag="m",
                                    name=f"yp{tch}_{half}")
                    nc.tensor.matmul(
                        yp, wvT[:, 128 * tch : 128 * tch + 128],
                        wo_sb[:, 512 * half : 512 * half + 512],
                        start=True, stop=False,
                    )
                    nc.tensor.matmul(
                        yp, ones128_r, bo8_h[:, 512 * half : 512 * half + 512],
                        start=False, stop=True, skip_group_check=True,
                    )
                    if half == 0:
                        nc.scalar.copy(
                            out=y_sb[:, 512 * half : 512 * half + 512], in_=yp)
                    else:
                        nc.vector.tensor_copy(
                            y_sb[:, 512 * half : 512 * half + 512], yp)
                if tch < 3:
                    nc.scalar.dma_start(
                        out=scr_d[128 * tch : 128 * tch + 128, :], in_=y_sb)
                else:
                    # SWDGE pre-generates the descriptors, so the final
                    # scratch write launches without the HWDGE latency.
                    nc.gpsimd.dma_start(
                        out=scr_d[128 * tch : 128 * tch + 128, :], in_=y_sb)

            # software-pipelined schedule: ktr leads qk by 1 stripe, wv by 2.
            stripes = [(bp, S) for bp in range(NBP) for S in range(NSTRIPE)]
            n = len(stripes)
            last = (NBP - 1, NSTRIPE - 1)
            for i in range(n + 4):
                if i < n:
                    bp, S = stripes[i]
                    if S % 4 == 0 and (bp, S // 4) != (0, 0):
                        load_unit(bp, S // 4)
                    ktr(bp, S)
                    if (bp, S) == last:
                        # the last stripe's K arrives well before its V:
                        # score+exp it immediately so only W@V and the
                        # extraction remain after the final bytes land.
                        qk(bp, S)
                if 2 <= i < n + 2:
                    bp1, S1 = stripes[i - 2]
                    if S1 == 0:
                        alloc_o(bp1)
                        if bp1 == NBP - 1:
                            # last pair: current-token first (it only needs
                            # stage-A data) so the post-stream drain is short.
                            current_token(bp1, opener=True)
                    if (bp1, S1) != last:
                        qk(bp1, S1)
                if i >= 4:
                    bp2, S2 = stripes[i - 4]
                    wv(bp2, S2)
                    if S2 == NSTRIPE - 1:
                        if bp2 != NBP - 1:
                            current_token(bp2, opener=False)
                        extract(bp2)
                        if bp2 % 2 == 1:
                            oproj(bp2 // 2)

        # ---------------- cross-core reduce + output ----------------
        nc.gpsimd.collective_compute(
            "ReduceScatter", mybir.AluOpType.add,
            [list(range(NCORES))],
            ins=[scr_d], outs=[rsb_d],
        )
        nc.gpsimd.dma_start(out=y_d, in_=rsb_d)



_NC_CACHE = None


def _get_nc():
    global _NC_CACHE
    if _NC_CACHE is None:
        _NC_CACHE = _build_kernel()
    return _NC_CACHE


def kernel(**inputs):
    x = np.ascontiguousarray(np.asarray(inputs["x"], dtype=np.float32))
    ck = np.ascontiguousarray(np.asarray(inputs["cache_k"], dtype=np.float32))
    cv = np.ascontiguousarray(np.asarray(inputs["cache_v"], dtype=np.float32))
    Wq = np.asarray(inputs["Wq"], dtype=np.float32)
    Wk = np.asarray(inputs["Wk"], dtype=np.float32)
    Wv = np.asarray(inputs["Wv"], dtype=np.float32)
    Wo = np.asarray(inputs["Wo"], dtype=np.float32)
    bq = np.asarray(inputs["bq"], dtype=np.float32)
    bv = np.asarray(inputs["bv"], dtype=np.float32)
    bo = np.ascontiguousarray(np.asarray(inputs["bo"], dtype=np.float32))

    nc = _get_nc()
    x2 = np.ascontiguousarray(x.reshape(TOK, D))
    in_maps = []
    for c in range(NCORES):
        sl = slice(DSL * c, DSL * (c + 1))
        in_maps.append({
            "x": x2,
            "cache_k": np.ascontiguousarray(
                ck[:, :, sl].reshape(B, KV // 128, 128, DSL).transpose(0, 2, 1, 3)),
            "cache_v": np.ascontiguousarray(
                cv[:, :, sl].reshape(B, KV // 128, 128, DSL).transpose(0, 2, 1, 3)),
            "Wq": np.ascontiguousarray(Wq[:, sl]),
            "Wk": np.ascontiguousarray(Wk[:, sl]),
            "Wv": np.ascontiguousarray(Wv[:, sl]),
            "Wo": np.ascontiguousarray(Wo[sl, :]),
            "bq": np.ascontiguousarray(bq[sl]),
            "bv": np.ascontiguousarray(bv[sl]),
            "bo": bo,
        })

    res = run_bass_kernel_spmd(nc, in_maps, core_ids=list(range(NCORES)))
    global _LAST_RESULT
    _LAST_RESULT = res
    y = np.concatenate(
        [r["y"].reshape(2, Q, D) for r in res.results], axis=0)
    return y


_LAST_RESULT = None


# revision 37
# speedup vs baseline: 1.1126x; 1.1126x over previous
"""Trainium2 Bass kernel for CachedMultiHeadAttention.

Problem: B=16, Q=32, KV=4096, D=1024, H=16 (DH=64), fp32 in/out.

Sharding: tensor-parallel over heads — each of the 8 cores owns 2 heads
(a contiguous 128-wide slice of D) for ALL 16 batches:
  - Wq/Wk/Wv column-sliced [1024, 128], Wo row-sliced [128, 1024],
    KV cache head-sliced [16, 4096, 128]; x replicated.
  - Each core computes a full-size partial y = wv_c @ Wo_c (+ bo/8);
    a ReduceScatter over the 8 cores sums the partials and hands core c
    tokens 64c..64c+64 (= batches 2c, 2c+1), which is exactly its output
    slice. Host-side work is pure slicing + concatenation.

Per-core dataflow (DMA-bound: ~202 µs of modeled 360 GB/s bandwidth):
  - KV cache streams from t=0 as fp16 via casting gpsimd (SWDGE) DMAs in
    4-stripe units (one 3-dim DMA per batch per tensor, 512 B src lines).
    5-unit tile rings give ~50 µs of prefetch slack.
  - Attention is fully 16-bit with scores built DIRECTLY in transposed
    orientation: stationary K^T chunk [dims, s] x moving block-diagonal
    q [dims, (2h x 32q)] -> scores^T [s, q-blocks]; exp straight off
    PSUM emits w^T; W@V runs with stationary NATURAL V [s, vd] x moving
    w^T -> o^T [vd, q-blocks], which is already the layout the output
    projection wants. No separate w transpose, no V shuffling.
  - Softmax denominators accumulate into 2 spare columns of the o^T
    accumulator via ones-moving matmuls (per-head row sums of w^T).
  - The stripe loop is software-pipelined: K^T transposes run one stripe
    ahead of QK, two ahead of W@V, so PE never waits on the Act/DVE
    copies between stages and the single DMA resource never idles.
  - Per batch-pair extraction: o^T -> (transpose) -> per-(head,q)
    normalize by reciprocal denominators -> (transpose back) -> wv^T.
  - Output projection per 128-token chunk: stationary wv^T x moving Wo
    row-slice (f32r N=512), bias via K=1 ones matmul; partial y goes to
    Internal DRAM scratch in bf16 so the ReduceScatter moves half the
    bytes; the rs output bounces through SBUF to the f32 output.
"""

import numpy as np

import concourse.bass as bass
import concourse.bacc as bacc
import concourse.mybir as mybir
import concourse.tile as tile
from concourse.bass_utils import run_bass_kernel_spmd
from concourse.masks import make_identity

F32 = mybir.dt.float32
F32R = mybir.dt.float32r
BF16 = mybir.dt.bfloat16
FP16 = mybir.dt.float16

B, Q, KV, D, H = 16, 32, 4096, 1024, 16
DH = D // H                     # 64
NCORES = 8
DSL = D // NCORES               # 128: per-core head-slice width (2 heads)
TOK = B * Q                     # 512 tokens, all batches
NBP = B // 2                    # 8 batch pairs
NSTRIPE = 8                     # stripes of 512 cached positions
STRIPE = 512
SCALE = float(DH) ** -0.5       # folded q*k scale (DH**-0.25 applied twice)
PREF = 5                        # KV prefetch ring depth in 4-stripe units
NO = 132                        # o^T accumulator: 128 o cols + 2 den + 2 pad


def _build_kernel():
    nc = bacc.Bacc(
        "TRN2",
        target_bir_lowering=False,
        debug=False,
        enable_asserts=False,
        num_devices=NCORES,
        dynamic_dma_scratch_size=32768,
    )

    x_d = nc.dram_tensor("x", [TOK, D], F32, kind="ExternalInput").ap()
    # KV shards arrive in [b, p, j, d] block layout (pure permutation done
    # while slicing on the host): DMA source runs are 16*128*4 B, so each
    # 4-stripe load is 128 descriptors instead of 1024 and the SWDGE ring
    # never throttles descriptor generation.
    ck_d = nc.dram_tensor("cache_k", [B, DSL, KV], F32,
                          kind="ExternalInput").ap()
    cv_d = nc.dram_tensor("cache_v", [B, 128, KV // 128, DSL], F32,
                          kind="ExternalInput").ap()
    wq_d = nc.dram_tensor("Wq", [D, DSL], F32, kind="ExternalInput").ap()
    wk_d = nc.dram_tensor("Wk", [D, DSL], F32, kind="ExternalInput").ap()
    wv_d = nc.dram_tensor("Wv", [D, DSL], F32, kind="ExternalInput").ap()
    wo_d = nc.dram_tensor("Wo", [DSL, D], F32R, kind="ExternalInput").ap()
    bq_d = nc.dram_tensor("bq", [DSL], F32R, kind="ExternalInput").ap()
    bv_d = nc.dram_tensor("bv", [DSL], F32R, kind="ExternalInput").ap()
    bo_d = nc.dram_tensor("bo", [D], F32, kind="ExternalInput").ap()
    scr_d = nc.dram_tensor("scratch", [TOK, D], BF16, kind="Internal").ap()
    rsb_d = nc.dram_tensor("rs_b", [TOK // NCORES, D], BF16, kind="Internal").ap()
    y_d = nc.dram_tensor("y", [TOK // NCORES, D], F32, kind="ExternalOutput").ap()

    with tile.TileContext(nc) as tc:
        _body(tc, x_d, ck_d, cv_d, wq_d, wk_d, wv_d, wo_d, bq_d, bv_d, bo_d,
              scr_d, rsb_d, y_d)
    nc.compile()
    return nc


def _body(tc, x_d, ck_d, cv_d, wq_d, wk_d, wv_d, wo_d, bq_d, bv_d, bo_d,
          scr_d, rsb_d, y_d):
    nc = tc.nc
    Exp = mybir.ActivationFunctionType.Exp

    ck_r = ck_d.rearrange("b p s -> p b s")      # [128 dims, 16, 4096]
    cv_r = cv_d.rearrange("b p j d -> p b j d")  # [128, 16, 32, 128]

    with (
        tc.tile_pool(name="consts", bufs=1) as consts,
        tc.tile_pool(name="knat", bufs=PREF) as knat_p,
        tc.tile_pool(name="vnat", bufs=PREF) as vnat_p,
        tc.tile_pool(name="wtp", bufs=8) as wt_p,
        tc.tile_pool(name="work", bufs=3) as work,
        tc.tile_pool(name="ysb", bufs=2) as ysb_p,
        tc.tile_pool(name="fin", bufs=1) as fin,
    ):
        ident_h = consts.tile([128, 128], FP16)
        make_identity(nc, ident_h)
        ident_f = consts.tile([128, 128], F32)
        make_identity(nc, ident_f)
        ident_r = consts.tile([128, 128], F32R)
        nc.vector.tensor_copy(ident_r, ident_f)
        ones2_h = consts.tile([128, 2], FP16)
        nc.vector.memset(ones2_h, 1.0)
        ones_tok_r = consts.tile([1, TOK], F32R)
        nc.vector.memset(ones_tok_r.bitcast(F32), 1.0)
        ones128_r = consts.tile([1, 128], BF16)
        nc.vector.memset(ones128_r, 1.0)

        bq_sb = consts.tile([1, DSL], F32R)
        bv_sb = consts.tile([1, DSL], F32R)
        bo_sb = consts.tile([1, D], F32)
        bo8_sb = consts.tile([1, D], F32R)          # bo / NCORES
        wo_sb = consts.tile([128, D], BF16)

        # stage-A persistent outputs
        xT = consts.tile([128, 8, TOK], FP16)       # [d-part, k-chunk, tok]
        qbd = consts.tile([128, B, DH], FP16)       # block-diag q per batch
        kTc = consts.tile([128, TOK], FP16)         # current-token K^T
        v_cur = consts.tile([Q, B, DSL], FP16)      # current-token V, natural
        nc.vector.memset(qbd, 0.0)                  # off-diag blocks stay 0
        wvT = consts.tile([128, TOK], BF16)         # normalized attn out^T

        # stage-A inputs; casting fp16 loads via gpsimd. The first KV unit
        # is issued ahead of x/W (see main loop) so the DMA engines start on
        # the big stream immediately; wo (f32r, non-cast) rides the scalar
        # queue.
        x_sb = consts.tile([128, 4, D], FP16)
        nc.scalar.dma_start(out=bq_sb, in_=bq_d.rearrange("(a d) -> a d", a=1))
        nc.scalar.dma_start(out=bv_sb, in_=bv_d.rearrange("(a d) -> a d", a=1))
        nc.scalar.dma_start(out=bo_sb, in_=bo_d.rearrange("(a d) -> a d", a=1))
        nc.vector.tensor_scalar_mul(bo8_sb, bo_sb, 1.0 / NCORES)
        bo8_h = consts.tile([1, D], BF16)
        nc.vector.tensor_copy(bo8_h, bo8_sb.bitcast(F32))
        nc.gpsimd.dma_start(out=wo_sb, in_=wo_d)
        wq_h = consts.tile([128, 8, DSL], FP16)
        wk_h = consts.tile([128, 8, DSL], FP16)
        wv_h = consts.tile([128, 8, DSL], FP16)
        nc.gpsimd.dma_start(out=wq_h, in_=wq_d.rearrange("(c p) d -> p c d", p=128))
        nc.gpsimd.dma_start(out=wk_h, in_=wk_d.rearrange("(c p) d -> p c d", p=128))
        nc.gpsimd.dma_start(out=wv_h, in_=wv_d.rearrange("(c p) d -> p c d", p=128))

        # first KV unit, issued ahead of x/W so the stream starts at once
        k_nat0 = knat_p.tile([128, 2, 4 * STRIPE], FP16, tag="knat", name="kn_0_0")
        v_nat0 = vnat_p.tile([128, 2, 16, DSL], FP16, tag="vnat", name="vn_0_0")
        for lo in range(0, 2048, 1024):
            for half, b in enumerate((0, 1)):
                nc.gpsimd.dma_start(out=k_nat0[:, half, lo : lo + 1024],
                                    in_=ck_r[:, b, lo : lo + 1024])
        for lo in range(0, 16, 8):
            for half, b in enumerate((0, 1)):
                nc.gpsimd.dma_start(out=v_nat0[:, half, lo : lo + 8, :],
                                    in_=cv_r[:, b, lo : lo + 8, :])
        nc.gpsimd.dma_start(out=x_sb, in_=x_d.rearrange("(c p) d -> p c d", p=128))

        # ---------------- stage A ----------------
        with tc.tile_pool(name="apsum", bufs=2, space="PSUM") as apsum:
            # x^T: per k-chunk, 4 fp16 transposes into one PSUM tile, 1 copy.
            for k in range(8):
                xt_ps = apsum.tile([128, 4, 128], FP16, tag="xt")
                for tch in range(4):
                    nc.tensor.matmul(
                        xt_ps[:, tch, :], x_sb[:, tch, 128 * k : 128 * k + 128],
                        ident_h, start=True, stop=True, is_transpose=True,
                    )
                if k % 2 == 0:
                    nc.scalar.copy(out=xT[:, k, :],
                                   in_=xt_ps.rearrange("p a b -> p (a b)"))
                else:
                    nc.vector.tensor_copy(xT[:, k, :],
                                          xt_ps.rearrange("p a b -> p (a b)"))

            # q^T [128 dims, 512 tok] (+bq), scattered to block-diag qbd
            qt_ps = apsum.tile([128, TOK], F32, tag="proj")
            for k in range(8):
                nc.tensor.matmul(qt_ps, wq_h[:, k, :], xT[:, k, :],
                                 start=(k == 0), stop=False)
            nc.tensor.matmul(qt_ps, bq_sb, ones_tok_r, start=False, stop=True,
                             skip_group_check=True)
            for b in range(B):
                if b % 2 == 0:
                    nc.scalar.copy(out=qbd[0:64, b, 0:Q],
                                   in_=qt_ps[0:64, Q * b : Q * b + Q])
                    nc.scalar.copy(out=qbd[64:128, b, Q : 2 * Q],
                                   in_=qt_ps[64:128, Q * b : Q * b + Q])
                else:
                    nc.vector.tensor_copy(qbd[0:64, b, 0:Q],
                                          qt_ps[0:64, Q * b : Q * b + Q])
                    nc.vector.tensor_copy(qbd[64:128, b, Q : 2 * Q],
                                          qt_ps[64:128, Q * b : Q * b + Q])

            # k_cur^T [128 dims, 512 tok]
            kt_ps = apsum.tile([128, TOK], F32, tag="proj")
            for k in range(8):
                nc.tensor.matmul(kt_ps, wk_h[:, k, :], xT[:, k, :],
                                 start=(k == 0), stop=(k == 7))
            nc.scalar.copy(out=kTc, in_=kt_ps)

            # v_cur per batch, natural [32 tok, 128 dims] on partitions 0:32
            # (stationary for the current-token W@V), bp-ascending order.
            for bb in range(B // 4):
                vn_ps = apsum.tile([Q, 4, DSL], F32, tag="vn")
                for i in range(4):
                    b = 4 * bb + i
                    for k in range(8):
                        nc.tensor.matmul(
                            vn_ps[:, i, :], xT[:, k, Q * b : Q * b + Q],
                            wv_h[:, k, :], start=(k == 0), stop=False)
                    nc.tensor.matmul(
                        vn_ps[:, i, :], ones_tok_r[:, 0:Q], bv_sb,
                        start=False, stop=True, skip_group_check=True)
                    dst = v_cur[:, b, :]
                    if i % 2 == 0:
                        nc.scalar.copy(out=dst, in_=vn_ps[:, i, :])
                    else:
                        nc.vector.tensor_copy(dst, vn_ps[:, i, :])

        # ---------------- main attention loop ----------------
        with (
            tc.tile_pool(name="scpsum", bufs=3, space="PSUM") as scpsum,
            tc.tile_pool(name="opsum", bufs=2, space="PSUM") as opsum,
            tc.tile_pool(name="mpsum", bufs=2, space="PSUM") as mpsum,
        ):
            o_tiles = {}
            wt_tiles = {}
            knat_tiles = {(0, 0): k_nat0}
            vnat_tiles = {(0, 0): v_nat0}

            def load_unit(bp, ss):
                """Casting DMAs for 4 stripes x both batches of a pair: K^T
                streams straight into its QK-stationary layout. The
                globally-last unit loads V per-stripe so the pipeline drain
                after the final bytes is one stripe deep, not four."""
                kt = knat_p.tile([128, 2, 4 * STRIPE], FP16, tag="knat",
                                 name=f"kn_{bp}_{ss}")
                v_nat = vnat_p.tile([128, 2, 16, DSL], FP16, tag="vnat",
                                    name=f"vn_{bp}_{ss}")
                for half, b in enumerate((2 * bp, 2 * bp + 1)):
                    nc.gpsimd.dma_start(
                        out=kt[:, half, :],
                        in_=ck_r[:, b, 2048 * ss : 2048 * ss + 2048])
                if not (bp == NBP - 1 and ss == 1):
                    for half, b in enumerate((2 * bp, 2 * bp + 1)):
                        nc.gpsimd.dma_start(
                            out=v_nat[:, half, :, :],
                            in_=cv_r[:, b, 16 * ss : 16 * ss + 16, :])
                else:
                    for lo in range(0, 16, 4):
                        for half, b in enumerate((2 * bp, 2 * bp + 1)):
                            nc.gpsimd.dma_start(
                                out=v_nat[:, half, lo : lo + 4, :],
                                in_=cv_r[:, b, 16 * ss + lo : 16 * ss + lo + 4, :])
                knat_tiles[(bp, ss)] = kt
                vnat_tiles[(bp, ss)] = v_nat

            def alloc_o(bp):
                o_ps = opsum.tile([128, NO], F32, tag="o", name=f"o_bp{bp}")
                o_tiles[bp] = o_ps

            def current_token(bp, opener=False):
                """Accumulation into o_ps[bp]: normally last (stop=True);
                for the final pair it runs first and opens the bank."""
                o_ps = o_tiles[bp]
                scc = mpsum.tile([Q, 128], F32, tag="m", name=f"scc{bp}")
                for half, b in enumerate((2 * bp, 2 * bp + 1)):
                    nc.tensor.matmul(
                        scc[:, 64 * half : 64 * half + 64],
                        kTc[:, Q * b : Q * b + Q], qbd[:, b, :],
                        start=True, stop=True,
                    )
                wtc = work.tile([Q, 128], FP16, tag="wtc")
                nc.scalar.activation(wtc, scc, Exp, scale=SCALE)
                for half, b in enumerate((2 * bp, 2 * bp + 1)):
                    nc.tensor.matmul(
                        o_ps[:, 64 * half : 64 * half + 64],
                        v_cur[:, b, :], wtc[:, 64 * half : 64 * half + 64],
                        start=(opener and half == 0), stop=not opener,
                        skip_group_check=True,
                    )
                nc.tensor.matmul(
                    o_ps[:, 128:130], wtc, ones2_h[0:Q, :],
                    start=False, stop=not opener, skip_group_check=True,
                )

            def qk(bp, S):
                """scores^T (+ denominator columns) for a stripe."""
                kt = knat_tiles[(bp, S // 4)]
                s4 = S % 4
                sc = scpsum.tile([128, 4, 128], F32, tag="sc")
                for jj in range(4):
                    for half, b in enumerate((2 * bp, 2 * bp + 1)):
                        nc.tensor.matmul(
                            sc[:, jj, 64 * half : 64 * half + 64],
                            kt[:, half,
                               512 * s4 + 128 * jj : 512 * s4 + 128 * jj + 128],
                            qbd[:, b, :],
                            start=True, stop=True,
                        )
                wt = wt_p.tile([128, 4, 128], FP16, tag="wt",
                               name=f"wt_{bp}_{S}")
                nc.scalar.activation(
                    wt.rearrange("p a b -> p (a b)"),
                    sc.rearrange("p a b -> p (a b)"),
                    Exp, scale=SCALE)
                # denominator accumulation depends only on K: issue it here
                # so the post-stream drain does not serialize behind it.
                o_ps = o_tiles[bp]
                for jj in range(4):
                    nc.tensor.matmul(
                        o_ps[:, 128:130], wt[:, jj, :], ones2_h,
                        start=(bp != NBP - 1 and S == 0 and jj == 0),
                        stop=(bp == NBP - 1 and S == NSTRIPE - 1 and jj == 3),
                        skip_group_check=True,
                    )
                wt_tiles[(bp, S)] = wt
                return wt

            def wv(bp, S):
                """o^T += V_nat-stationary stripe."""
                o_ps = o_tiles[bp]
                v_nat = vnat_tiles[(bp, S // 4)]
                s4 = S % 4
                wt = wt_tiles.pop((bp, S))
                jjs = range(4)
                # exactly ONE start per o_ps bank (start marks the whole 2KB
                # zero-region pending; other regions' first writes land fresh
                # via the pending flag and must NOT re-assert start). For the
                # final pair the opener is the early current-token matmul.
                closer_here = bp == NBP - 1 and S == NSTRIPE - 1
                for jj in jjs:
                    for half in range(2):
                        nc.tensor.matmul(
                            o_ps[:, 64 * half : 64 * half + 64],
                            v_nat[:, half, 4 * s4 + jj, :],
                            wt[:, jj, 64 * half : 64 * half + 64],
                            start=False,
                            stop=(closer_here and jj == 3),
                            skip_group_check=True,
                        )

            def extract(bp):
                """normalize o^T by per-(head,q) denominators -> wvT."""
                o_ps = o_tiles.pop(bp)
                o_sb = work.tile([128, 128], F32R, tag="osb")
                nc.scalar.copy(out=o_sb, in_=o_ps[:, 0:128])
                rec = work.tile([128, 1], F32, tag="rec")
                nc.vector.reciprocal(rec, o_ps[:, 128:129])
                t2 = mpsum.tile([128, 128], F32R, tag="m", name=f"t2_{bp}")
                nc.tensor.matmul(t2, o_sb, ident_r,
                                 start=True, stop=True, is_transpose=True)
                onrm = work.tile([128, 128], F32R, tag="onrm")
                nc.vector.tensor_scalar_mul(onrm, t2, rec)
                t3 = mpsum.tile([128, 128], F32R, tag="m", name=f"t3_{bp}")
                nc.tensor.matmul(t3, onrm, ident_r,
                                 start=True, stop=True, is_transpose=True)
                for half, b in enumerate((2 * bp, 2 * bp + 1)):
                    if half == 0:
                        nc.scalar.copy(
                            out=wvT[0:64, Q * b : Q * b + Q],
                            in_=t3[0:64, 64 * half : 64 * half + Q])
                        nc.scalar.copy(
                            out=wvT[64:128, Q * b : Q * b + Q],
                            in_=t3[64:128, 64 * half + Q : 64 * half + 2 * Q])
                    else:
                        nc.vector.tensor_copy(
                            wvT[0:64, Q * b : Q * b + Q],
                            t3[0:64, 64 * half : 64 * half + Q])
                        nc.vector.tensor_copy(
                            wvT[64:128, Q * b : Q * b + Q],
                            t3[64:128, 64 * half + Q : 64 * half + 2 * Q])

            def oproj(tch):
                """output projection + bias for one 128-token chunk."""
                y_sb = ysb_p.tile([128, D], BF16, tag="ysb")
                for half in range(2):
                    yp = mpsum.tile([128, 512], F32, tag="m",
                                    name=f"yp{tch}_{half}")
                    nc.tensor.matmul(
                        yp, wvT[:, 128 * tch : 128 * tch + 128],
                        wo_sb[:, 512 * half : 512 * half + 512],
                        start=True, stop=False,
                    )
                    nc.tensor.matmul(
                        yp, ones128_r, bo8_h[:, 512 * half : 512 * half + 512],
                        start=False, stop=True, skip_group_check=True,
                    )
                    if half == 0:
                        nc.scalar.copy(
                            out=y_sb[:, 512 * half : 512 * half + 512], in_=yp)
                    else:
                        nc.vector.tensor_copy(
                            y_sb[:, 512 * half : 512 * half + 512], yp)
                if tch < 3:
                    nc.scalar.dma_start(
                        out=scr_d[128 * tch : 128 * tch + 128, :], in_=y_sb)
                else:
                    # SWDGE pre-generates the descriptors, so the final
                    # scratch write launches without the HWDGE latency.
                    nc.gpsimd.dma_start(
                        out=scr_d[128 * tch : 128 * tch + 128, :], in_=y_sb)

            # software-pipelined schedule: ktr leads qk by 1 stripe, wv by 2.
            stripes = [(bp, S) for bp in range(NBP) for S in range(NSTRIPE)]
            n = len(stripes)
            for i in range(n + 3):
                if i < n:
                    bp, S = stripes[i]
                    if S % 4 == 0 and (bp, S // 4) != (0, 0):
                        load_unit(bp, S // 4)
                if 1 <= i < n + 1:
                    bp1, S1 = stripes[i - 1]
                    if S1 == 0:
                        alloc_o(bp1)
                        if bp1 == NBP - 1:
                            # last pair: current-token first (it only needs
                            # stage-A data) so the post-stream drain is short.
                            current_token(bp1, opener=True)
                    qk(bp1, S1)
                if i >= 3:
                    bp2, S2 = stripes[i - 3]
                    wv(bp2, S2)
                    if S2 == NSTRIPE - 1:
                        if bp2 != NBP - 1:
                            current_token(bp2, opener=False)
                        extract(bp2)
                        if bp2 % 2 == 1:
                            oproj(bp2 // 2)

        # ---------------- cross-core reduce + output ----------------
        nc.gpsimd.collective_compute(
            "ReduceScatter", mybir.AluOpType.add,
            [list(range(NCORES))],
            ins=[scr_d], outs=[rsb_d],
        )
        nc.gpsimd.dma_start(out=y_d, in_=rsb_d)



_NC_CACHE = None


def _get_nc():
    global _NC_CACHE
    if _NC_CACHE is None:
        _NC_CACHE = _build_kernel()
    return _NC_CACHE


def kernel(**inputs):
    x = np.ascontiguousarray(np.asarray(inputs["x"], dtype=np.float32))
    ck = np.ascontiguousarray(np.asarray(inputs["cache_k"], dtype=np.float32))
    cv = np.ascontiguousarray(np.asarray(inputs["cache_v"], dtype=np.float32))
    Wq = np.asarray(inputs["Wq"], dtype=np.float32)
    Wk = np.asarray(inputs["Wk"], dtype=np.float32)
    Wv = np.asarray(inputs["Wv"], dtype=np.float32)
    Wo = np.asarray(inputs["Wo"], dtype=np.float32)
    bq = np.asarray(inputs["bq"], dtype=np.float32)
    bv = np.asarray(inputs["bv"], dtype=np.float32)
    bo = np.ascontiguousarray(np.asarray(inputs["bo"], dtype=np.float32))

    nc = _get_nc()
    x2 = np.ascontiguousarray(x.reshape(TOK, D))
    in_maps = []
    for c in range(NCORES):
        sl = slice(DSL * c, DSL * (c + 1))
        in_maps.append({
            "x": x2,
            "cache_k": np.ascontiguousarray(ck[:, :, sl].transpose(0, 2, 1)),
            "cache_v": np.ascontiguousarray(
                cv[:, :, sl].reshape(B, KV // 128, 128, DSL).transpose(0, 2, 1, 3)),
            "Wq": np.ascontiguousarray(Wq[:, sl]),
            "Wk": np.ascontiguousarray(Wk[:, sl]),
            "Wv": np.ascontiguousarray(Wv[:, sl]),
            "Wo": np.ascontiguousarray(Wo[sl, :]),
            "bq": np.ascontiguousarray(bq[sl]),
            "bv": np.ascontiguousarray(bv[sl]),
            "bo": bo,
        })

    res = run_bass_kernel_spmd(nc, in_maps, core_ids=list(range(NCORES)))
    global _LAST_RESULT
    _LAST_RESULT = res
    y = np.concatenate(
        [r["y"].reshape(2, Q, D) for r in res.results], axis=0)
    return y


_LAST_RESULT = None
